# revision 17
# baseline (speedup 1.0000x reference)
"""Trainium2 Bass kernel: JointSpatioTemporalAttention, bf16 matmuls, 8-core SPMD.

Sharding: core c = (b, g) with b = c // 4 (batch), g = c % 4 (KV group).
Each core computes q-heads [4g, 4g+4) and kv-head g for batch b end-to-end
(QKV proj -> RMSNorm -> 3D RoPE -> attention -> partial out-proj), returning
the partial y^T = Wo[256g:256g+256, :].T @ O_norm^T.  Host sums the 4 group
partials per batch and adds bo.  No on-device collectives.

v2 layout strategy (all matmul operands bf16; PSUM accumulation fp32):
  - QKV projection computed in NATURAL orientation: stationary = x^T block
    [128 dchunk, 128 tokens], moving = fused W_qkv [128, 384] -> psum
    [128 tokens, 384].  Bias added via an extra ones-row matmul (contraction
    row of 1s x bias row).  No forward transposes needed.
  - head_dim columns of Q/K are PERMUTED (pair-interleaved per rope segment)
    so rot_half becomes a swap of adjacent column pairs: one strided-AP
    multiply.  Scores are invariant to a shared q/k column permutation.
  - RMSNorm scale applied AFTER rope (they commute; qn_w/kn_w folded into
    cos/sin tables on host).
  - Attention S^T per (head, token-half): lhsT = K^T chunk [64, 128],
    rhs = Q^T [64, 1024] -> psum S^T [128 keys, 1024 queries]; ACT exp
    (scale 1/8) -> P^T bf16.
  - PV reoriented queries-on-partitions: stationary = P^T block [128 keys,
    128 queries], moving = V||ones [128, 65] -> accumulate O' [128 q, 65]
    over 16 key chunks; denominator lands in col 64; DVE normalizes with a
    per-partition reciprocal into natural O bf16.
  - O transposed back to O^T via PE transposes for the out-proj.
"""

import os
import sys
from contextlib import ExitStack

import numpy as np

for _p in ("/opt/trn_rl_repo",):
    if _p not in sys.path:
        sys.path.append(_p)

import concourse.bass as bass  # noqa: E402
import concourse.mybir as mybir  # noqa: E402
import concourse.tile as tile  # noqa: E402
from concourse import bacc  # noqa: E402
from concourse.bass_utils import run_bass_kernel_spmd  # noqa: E402

import ml_dtypes  # noqa: E402

F32 = mybir.dt.float32
BF16 = mybir.dt.bfloat16
AF = mybir.ActivationFunctionType
ALU = mybir.AluOpType
AX = mybir.AxisListType

P = 128
B = 2
N = 2048          # tokens = 8*16*16
D = 1024
HD = 64           # head dim
NHEADS = 16
NKV = 4
CQ = 256          # q cols per core (4 heads)
CK = 64           # k/v cols per core (1 kv head)
CQK = CQ + CK     # 320
CQKV = CQK + CK   # 384: fused q+k+v natural tile width
NT = N // P       # 16 token tiles
KD = D // P       # 8 contraction chunks
T_, H_, W_ = 8, 16, 16
THETA = 10000.0
EPS = 1e-6
NCORES = 8

_PROGRAM = None  # (nc, input_names) cache
LAST_RESULTS = None  # BassKernelResults of the last run (for test harness)


def _emit(ctx: ExitStack, tc: "tile.TileContext"):
    nc = tc.nc
    f = F32
    bf = BF16

    xT = nc.dram_tensor("xT", [D, N], bf, kind="ExternalInput").ap()
    wqkv = nc.dram_tensor("wqkv", [D, CQKV], bf, kind="ExternalInput").ap()
    biasd = nc.dram_tensor("biasv", [CQKV], bf, kind="ExternalInput").ap()
    wo = nc.dram_tensor("wo", [CQ, D], bf, kind="ExternalInput").ap()
    cosd = nc.dram_tensor("cos5", [N, CQK], bf, kind="ExternalInput").ap()
    sind = nc.dram_tensor("sin5", [N, CQK], bf, kind="ExternalInput").ap()
    identd = nc.dram_tensor("ident", [P, P], bf, kind="ExternalInput").ap()
    yT = nc.dram_tensor("yT", [D, N], f, kind="ExternalOutput").ap()

    # ---------------- persistent pools ----------------
    constp = ctx.enter_context(tc.tile_pool(name="const", bufs=1))
    ident = constp.tile([P, P], bf, tag="ident")
    nc.sync.dma_start(out=ident[:], in_=identd)
    epsb = constp.tile([P, 1], f, tag="epsb")
    nc.vector.memset(epsb[:], EPS)
    ones_r = constp.tile([1, P], bf, tag="ones_r")
    nc.vector.memset(ones_r[:], 1.0)
    bias_sb = constp.tile([1, CQKV], bf, tag="bias_sb")
    nc.sync.dma_start(out=bias_sb[:], in_=biasd.rearrange("(a c) -> a c", a=1))
    wo_sb = constp.tile([P, 2, D], bf, tag="wo")
    wo_t = wo.rearrange("(k p) d -> k p d", p=P)
    for k2 in range(2):
        nc.sync.dma_start(out=wo_sb[:, k2, :], in_=wo_t[k2])

    persist = ctx.enter_context(tc.tile_pool(name="persist", bufs=1))
    # QT2[:, i, :] = Q^T for head pair i (head 2i at partitions 0:64, 2i+1 at 64:128)
    QT2 = persist.tile([P, 2, N], bf, tag="QT2")
    # K^T duplicated on partitions [0:64] and [64:128]
    kT2 = persist.tile([P, N], bf, tag="kT2")
    # natural qkv + ones column: [.., 0:320]=qk (scratch), [.., 320:384]=v,
    # [.., 384]=1.0 -> PV moving operand is qnat[:, mc, 320:385]
    qnat = persist.tile([P, NT, CQKV + 1], bf, tag="qnat")
    nc.vector.memset(qnat[:, :, CQKV:CQKV + 1], 1.0)
    # O natural [tokens(tile-major), 4 heads * 64]
    O_nat = persist.tile([P, NT, CQ], bf, tag="O_nat")
    # O^T for out-proj: [:, k2, :] = rows of heads (2*k2, 2*k2+1)
    oT = persist.tile([P, 2, N], bf, tag="oT")
    # Schraudolph fast-exp constant: bf16 bits of exp(x) ~ int16(x*A + B)
    schrB = persist.tile([P, 1], f, tag="schrB")
    nc.vector.memset(schrB[:], 127.0 * 128.0 - 5.5)

    # ================ phase A: proj + norm + rope + transposes ================
    with tc.tile_pool(name="xw", bufs=1) as xwp, \
         tc.tile_pool(name="cs", bufs=1) as csp, \
         tc.tile_pool(name="psA", bufs=4, space="PSUM") as psA, \
         tc.tile_pool(name="psT", bufs=2, space="PSUM") as psT, \
         tc.tile_pool(name="nat", bufs=3) as natp, \
         tc.tile_pool(name="sm", bufs=4) as smp:

        xT_sb = xwp.tile([P, KD, N], bf, tag="xT")
        xT_t = xT.rearrange("(k p) n -> k p n", p=P)
        for k in range(KD):
            nc.sync.dma_start(out=xT_sb[:, k, :], in_=xT_t[k])
        w_sb = xwp.tile([P, KD, CQKV], bf, tag="wqkv")
        w_t = wqkv.rearrange("(k p) m -> k p m", p=P)
        for k in range(KD):
            nc.sync.dma_start(out=w_sb[:, k, :], in_=w_t[k])

        cosd_t = cosd.rearrange("(t p) c -> t p c", p=P)
        sind_t = sind.rearrange("(t p) c -> t p c", p=P)

        pas = {}

        def emit_proj(i):
            nsl = slice(i * P, (i + 1) * P)
            pa = psA.tile([P, CQKV], f, tag="pa", name=f"pa{i}")
            for k in range(KD):
                nc.tensor.matmul(
                    pa[:],
                    lhsT=xT_sb[:, k, nsl],
                    rhs=w_sb[:, k, :],
                    start=(k == 0),
                    stop=False,
                )
            nc.tensor.matmul(
                pa[:], lhsT=ones_r[:], rhs=bias_sb[:], start=False, stop=True,
            )
            pas[i] = pa

        def emit_post(i):
            nsl = slice(i * P, (i + 1) * P)
            pa = pas.pop(i)
            qk = qnat[:, i, 0:CQK]
            nc.scalar.copy(qnat[:, i, 0:CQKV], pa[:])    # ACT: PSUM -> SBUF bf16

            cos_t = csp.tile([P, CQK], bf, tag="cos", bufs=6)
            nc.sync.dma_start(out=cos_t[:], in_=cosd_t[i])
            sin_t = csp.tile([P, CQK], bf, tag="sin", bufs=6)
            nc.sync.dma_start(out=sin_t[:], in_=sind_t[i])

            # ---- RMS stats on raw qk (norm applied after rope) ----
            sq = smp.tile([P, CQK], bf, tag="sq")
            nc.gpsimd.tensor_tensor(sq[:], qk, qk, op=ALU.mult)  # Pool
            ssum = smp.tile([P, 5], f, tag="ssum")
            nc.vector.tensor_reduce(
                ssum[:], sq.rearrange("p (h d) -> p h d", d=HD), axis=AX.X, op=ALU.add
            )
            rms = smp.tile([P, 5], f, tag="rms")
            nc.scalar.activation(rms[:], ssum[:], AF.Sqrt, bias=epsb[:], scale=1.0 / HD)
            rinv = smp.tile([P, 5], f, tag="rinv")
            nc.vector.reciprocal(rinv[:], rms[:])

            # ---- RoPE in pair-interleaved layout (fp32 intermediates) ----
            qkc = smp.tile([P, CQK], f, tag="qkc")
            if i % 2 == 0:
                nc.gpsimd.tensor_tensor(qkc[:], qk, cos_t[:], op=ALU.mult)
            else:
                nc.vector.tensor_tensor(qkc[:], qk, cos_t[:], op=ALU.mult)
            rs = smp.tile([P, CQK], f, tag="rs")
            qk_sw = qk.rearrange("p (j s) -> p j s", s=2)[:, :, ::-1]
            nc.vector.tensor_tensor(rs[:], qk_sw, sin_t[:], op=ALU.mult)
            qka = smp.tile([P, CQK], f, tag="qka")
            if i % 2 == 0:
                nc.vector.tensor_tensor(qka[:], qkc[:], rs[:], op=ALU.add)
            else:
                nc.gpsimd.tensor_tensor(qka[:], qkc[:], rs[:], op=ALU.add)
            qkf = natp.tile([P, CQK], bf, tag="qkf")
            qkf5 = qkf.rearrange("p (h d) -> p h d", d=HD)
            nc.vector.tensor_mul(
                qkf5, qka.rearrange("p (h d) -> p h d", d=HD),
                rinv.to_broadcast((P, 5, HD)),
            )

            # ---- transpose back: q pair (ACT copy) + k dup (DVE copy) ----
            pt2 = psT.tile([P, 3, P], bf, tag="pt2")
            nc.tensor.transpose(pt2[:, 0, :], qkf[:, 0:P], ident[:])
            nc.tensor.transpose(pt2[:, 1, :], qkf[:, P:2 * P], ident[:])
            nc.tensor.transpose(pt2[0:CK, 2, :], qkf[:, 2 * P:2 * P + CK], ident[:])
            nc.tensor.transpose(pt2[CK:2 * CK, 2, :], qkf[:, 2 * P:2 * P + CK], ident[:])
            nc.scalar.copy(QT2[:, :, nsl], pt2[:, 0:2, :])
            nc.scalar.copy(kT2[:, nsl], pt2[:, 2, :])

        LAG = 3
        for i in range(NT):
            emit_proj(i)
            if i >= LAG:
                emit_post(i - LAG)
        for i in range(NT - LAG, NT):
            emit_post(i)

    # ================ phase B: attention ================
    scale = float(HD) ** -0.5
    kdebug = bool(os.environ.get("KDEBUG"))
    if kdebug:
        dP16 = nc.dram_tensor("dP16", [NT, P, 1024], BF16, kind="ExternalOutput").ap()
    with tc.tile_pool(name="psS", bufs=2, space="PSUM") as psS, \
         tc.tile_pool(name="psO", bufs=1, space="PSUM") as psO, \
         tc.tile_pool(name="psN", bufs=2, space="PSUM") as psN, \
         tc.tile_pool(name="pT", bufs=3) as pTp, \
         tc.tile_pool(name="ot", bufs=2) as otp, \
         tc.tile_pool(name="rr", bufs=2) as rrp:
        for hh in range(2):          # query half
            nof = hh * 1024
            for h in range(4):       # head
                qrow = (h % 2) * CK
                # O'^T accumulator [65, 1024]: V||ones stationary (cheap
                # amortized weight load), P^T moving 512 cols per matmul
                ops = psO.tile([HD + 1, 1024], f, tag="psO")
                pT_ring = []
                def emit_pv(mc):
                    pTt = pT_ring[mc]
                    for sg in range(2):
                        nc.tensor.matmul(
                            ops[:, sg * 512:(sg + 1) * 512],
                            lhsT=qnat[:, mc, CQK:CQKV + 1],
                            rhs=pTt[:, sg * 512:(sg + 1) * 512],
                            start=(mc == 0),
                            stop=(mc == NT - 1),
                        )
                for mc in range(NT):  # key chunk
                    msl = slice(mc * P, (mc + 1) * P)
                    sps = psS.tile([P, 1024], f, tag="psS")
                    for sg in range(2):
                        nc.tensor.matmul(
                            sps[:, sg * 512:(sg + 1) * 512],
                            lhsT=kT2[qrow:qrow + CK, msl],
                            rhs=QT2[qrow:qrow + CK, h // 2,
                                    nof + sg * 512:nof + (sg + 1) * 512],
                            start=True,
                            stop=True,
                        )
                    pTt = pTp.tile([P, 1024], bf, tag="pT", name=f"pT{mc % 3}")
                    pT_ring.append(pTt)
                    # exp split: queries 0:512 exact on ACT, 512:1024 via
                    # DVE Schraudolph fast-exp (bf16 bits = int16(s*A + B))
                    nc.scalar.activation(pTt[:, 0:512], sps[:, 0:512],
                                         AF.Exp, scale=scale)
                    nc.vector.scalar_tensor_tensor(
                        pTt[:, 512:1024].bitcast(mybir.dt.int16),
                        sps[:, 512:1024],
                        float(128.0 * 1.4426950408889634) * scale,
                        schrB.to_broadcast((P, 512)),
                        op0=ALU.mult, op1=ALU.add,
                    )
                    if kdebug and hh == 0 and h == 0:
                        nc.sync.dma_start(out=dP16[mc], in_=pTt[:])
                    if mc >= 1:
                        emit_pv(mc - 1)
                emit_pv(NT - 1)
                # O'^T -> SBUF bf16, transpose to natural, normalize per-q
                o_tmp = otp.tile([HD + 1, 1024], bf, tag="o_tmp")
                nc.vector.tensor_copy(o_tmp[:], ops[:])
                ptn = psN.tile([P, 8, HD + 2], bf, tag="ptn")
                for qb in range(8):
                    nc.tensor.transpose(
                        ptn[:, qb, 0:HD + 1],
                        o_tmp[:, qb * P:(qb + 1) * P],
                        ident[0:HD + 1, 0:HD + 1],
                    )
                rin = rrp.tile([P, 8], f, tag="rin")
                nc.vector.reciprocal(rin[:], ptn[:, :, HD])
                nc.vector.tensor_mul(
                    O_nat[:, hh * 8:(hh + 1) * 8, h * HD:(h + 1) * HD],
                    ptn[:, :, 0:HD],
                    rin.to_broadcast((P, 8, HD)),
                )

    # ================ phase B2: O_nat -> O^T ================
    with tc.tile_pool(name="psU", bufs=2, space="PSUM") as psU:
        for i in range(NT):
            ptu = psU.tile([P, 2, P], bf, tag="ptu")
            for k2 in range(2):
                nc.tensor.transpose(
                    ptu[:, k2, :], O_nat[:, i, k2 * P:(k2 + 1) * P], ident[:]
                )
            nc.vector.tensor_copy(oT[:, :, i * P:(i + 1) * P], ptu[:])

    if os.environ.get("KDEBUG"):
        for nm, t in (("dQT2", QT2), ("dkT2", kT2), ("dvnat", v_nat),
                      ("dOnat", O_nat), ("doT", oT)):
            shp = [P] + list(t.shape[1:])
            dd = nc.dram_tensor(nm, shp, BF16, kind="ExternalOutput").ap()
            nc.sync.dma_start(out=dd, in_=t[:])

    # ================ phase C: partial out-proj y^T = Wo_g.T @ O_norm^T ======
    with tc.tile_pool(name="psY", bufs=2, space="PSUM") as psY, \
         tc.tile_pool(name="ysb", bufs=2) as ysbp:
        yT_t = yT.rearrange("(t p) n -> t p n", p=P)
        for mt in range(8):
            yps = psY.tile([P, N], f, tag="psY")
            for k2 in range(2):
                for sg in range(4):
                    nc.tensor.matmul(
                        yps[:, sg * 512:(sg + 1) * 512],
                        lhsT=wo_sb[:, k2, mt * P:(mt + 1) * P],
                        rhs=oT[:, k2, sg * 512:(sg + 1) * 512],
                        start=(k2 == 0),
                        stop=(k2 == 1),
                    )
            ysb = ysbp.tile([P, N], f, tag="ysb")
            if mt % 2 == 0:
                nc.vector.tensor_copy(ysb[:], yps[:])
            else:
                nc.scalar.copy(ysb[:], yps[:])
            nc.sync.dma_start(out=yT_t[mt], in_=ysb[:])


def _build_program():
    global _PROGRAM
    if _PROGRAM is not None:
        return _PROGRAM
    nc = bacc.Bacc(
        "TRN2",
        target_bir_lowering=False,
        debug=False,
        enable_asserts=False,
        num_devices=NCORES,
    )
    with tile.TileContext(nc) as tc:
        with ExitStack() as ctx:
            _emit(ctx, tc)
    nc.finalize()
    _PROGRAM = nc
    return nc


# ---------------- host-side RoPE/scale table construction ----------------

def _rope_cs(n, d):
    # bit-identical to reference._rope_cs
    inv = 1.0 / (THETA ** (np.arange(0, d, 2, dtype=np.float32) / d))
    fr = np.arange(n, dtype=np.float32)[:, None] * inv[None, :]
    emb = np.concatenate([fr, fr], axis=-1)
    return np.cos(emb), np.sin(emb)


def _perm():
    dt = HD // 4          # 16
    dh = HD // 4          # 16
    dw = HD - dt - dh     # 32
    perm = np.empty(HD, np.int64)
    for off, sz in ((0, dt), (dt, dh), (dt + dh, dw)):
        m = sz // 2
        for j in range(m):
            perm[off + 2 * j] = off + j
            perm[off + 2 * j + 1] = off + m + j
    return perm


def _host_tables(qn_w, kn_w):
    dt = HD // 4
    dh = HD // 4
    dw = HD - dt - dh
    cos_t, sin_t = _rope_cs(T_, dt)
    cos_h, sin_h = _rope_cs(H_, dh)
    cos_w, sin_w = _rope_cs(W_, dw)
    tt = np.repeat(np.arange(T_), H_ * W_)
    hh = np.tile(np.repeat(np.arange(H_), W_), T_)
    ww = np.tile(np.arange(W_), T_ * H_)
    cos = np.empty((N, HD), np.float32)
    sin = np.empty((N, HD), np.float32)
    cos[:, 0:dt] = cos_t[tt]
    cos[:, dt:dt + dh] = cos_h[hh]
    cos[:, dt + dh:] = cos_w[ww]
    sin[:, 0:dt] = sin_t[tt]
    sin[:, dt:dt + dh] = sin_h[hh]
    sin[:, dt + dh:] = sin_w[ww]

    perm = _perm()
    cosP = cos[:, perm]
    sgn = np.empty(HD, np.float32)
    sgn[0::2] = -1.0
    sgn[1::2] = 1.0
    sgnsinP = sin[:, perm] * sgn[None, :]

    def fold(w):
        wp = np.asarray(w, np.float32)[perm]
        swp = wp.reshape(-1, 2)[:, ::-1].reshape(-1)
        return cosP * wp[None, :], sgnsinP * swp[None, :]

    cos_q, sin_q = fold(qn_w)
    cos_k, sin_k = fold(kn_w)
    cos5 = np.concatenate([np.tile(cos_q, (1, 4)), cos_k], axis=1)
    sin5 = np.concatenate([np.tile(sin_q, (1, 4)), sin_k], axis=1)
    return np.ascontiguousarray(cos5), np.ascontiguousarray(sin5)


def _bf16(a):
    return np.asarray(a, np.float32).astype(ml_dtypes.bfloat16)


def kernel(**inputs):
    global LAST_RESULTS
    x = np.asarray(inputs["x"], np.float32)
    Wq = np.asarray(inputs["Wq"], np.float32)
    Wk = np.asarray(inputs["Wk"], np.float32)
    Wv = np.asarray(inputs["Wv"], np.float32)
    Wo = np.asarray(inputs["Wo"], np.float32)
    bq = np.asarray(inputs["bq"], np.float32)
    bk = np.asarray(inputs["bk"], np.float32)
    bv = np.asarray(inputs["bv"], np.float32)
    bo = np.asarray(inputs["bo"], np.float32)
    qn_w = np.asarray(inputs["qn_w"], np.float32)
    kn_w = np.asarray(inputs["kn_w"], np.float32)

    assert x.shape == (B, N, D), x.shape
    cos5, sin5 = _host_tables(qn_w, kn_w)
    perm = _perm()

    nc = _build_program()
    in_maps = []
    xT_b = [np.ascontiguousarray(_bf16(x[b]).T) for b in range(B)]
    identb = np.eye(P, dtype=ml_dtypes.bfloat16)
    cos5b = _bf16(cos5)
    sin5b = _bf16(sin5)
    for c in range(NCORES):
        b, g = c // 4, c % 4
        wq_g = Wq[:, g * CQ:(g + 1) * CQ].reshape(D, 4, HD)[:, :, perm].reshape(D, CQ)
        wk_g = Wk[:, g * CK:(g + 1) * CK][:, perm]
        wv_g = Wv[:, g * CK:(g + 1) * CK]
        wqkv = np.concatenate([wq_g, wk_g, wv_g], axis=1)
        bq_g = bq[g * CQ:(g + 1) * CQ].reshape(4, HD)[:, perm].reshape(CQ)
        bk_g = bk[g * CK:(g + 1) * CK][perm]
        bias = np.concatenate([bq_g, bk_g, bv[g * CK:(g + 1) * CK]])
        in_maps.append({
            "xT": xT_b[b],
            "wqkv": np.ascontiguousarray(_bf16(wqkv)),
            "biasv": np.ascontiguousarray(_bf16(bias)),
            "wo": np.ascontiguousarray(_bf16(Wo[g * CQ:(g + 1) * CQ, :])),
            "cos5": cos5b,
            "sin5": sin5b,
            "ident": identb,
        })

    res = run_bass_kernel_spmd(nc, in_maps, list(range(NCORES)))
    LAST_RESULTS = res
    out = np.empty((B, N, D), np.float32)
    for b in range(B):
        acc = res.results[4 * b]["yT"].astype(np.float32)
        for g in range(1, 4):
            acc = acc + res.results[4 * b + g]["yT"]
        out[b] = acc.T + bo[None, :]
    return out


if __name__ == "__main__":
    # smoke: build only
    nc = _build_program()
    print("built ok")


# revision 18
# speedup vs baseline: 1.2467x; 1.2467x over previous
"""Trainium2 Bass kernel: JointSpatioTemporalAttention, bf16 matmuls, 8-core SPMD.

Sharding: core c = (b, g) with b = c // 4 (batch), g = c % 4 (KV group).
Each core computes q-heads [4g, 4g+4) and kv-head g for batch b end-to-end
(QKV proj -> RMSNorm -> 3D RoPE -> attention -> partial out-proj), returning
the partial y^T = Wo[256g:256g+256, :].T @ O_norm^T.  Host sums the 4 group
partials per batch and adds bo.  No on-device collectives.

v2 layout strategy (all matmul operands bf16; PSUM accumulation fp32):
  - QKV projection computed in NATURAL orientation: stationary = x^T block
    [128 dchunk, 128 tokens], moving = fused W_qkv [128, 384] -> psum
    [128 tokens, 384].  Bias added via an extra ones-row matmul (contraction
    row of 1s x bias row).  No forward transposes needed.
  - head_dim columns of Q/K are PERMUTED (pair-interleaved per rope segment)
    so rot_half becomes a swap of adjacent column pairs: one strided-AP
    multiply.  Scores are invariant to a shared q/k column permutation.
  - RMSNorm scale applied AFTER rope (they commute; qn_w/kn_w folded into
    cos/sin tables on host).
  - Attention S^T per (head, token-half): lhsT = K^T chunk [64, 128],
    rhs = Q^T [64, 1024] -> psum S^T [128 keys, 1024 queries]; ACT exp
    (scale 1/8) -> P^T bf16.
  - PV reoriented queries-on-partitions: stationary = P^T block [128 keys,
    128 queries], moving = V||ones [128, 65] -> accumulate O' [128 q, 65]
    over 16 key chunks; denominator lands in col 64; DVE normalizes with a
    per-partition reciprocal into natural O bf16.
  - O transposed back to O^T via PE transposes for the out-proj.
"""

import os
import sys
from contextlib import ExitStack

import numpy as np

for _p in ("/opt/trn_rl_repo",):
    if _p not in sys.path:
        sys.path.append(_p)

import concourse.bass as bass  # noqa: E402
import concourse.mybir as mybir  # noqa: E402
import concourse.tile as tile  # noqa: E402
from concourse import bacc  # noqa: E402
from concourse.bass_utils import run_bass_kernel_spmd  # noqa: E402

import ml_dtypes  # noqa: E402

F32 = mybir.dt.float32
BF16 = mybir.dt.bfloat16
AF = mybir.ActivationFunctionType
ALU = mybir.AluOpType
AX = mybir.AxisListType

P = 128
B = 2
N = 2048          # tokens = 8*16*16
D = 1024
HD = 64           # head dim
NHEADS = 16
NKV = 4
CQ = 256          # q cols per core (4 heads)
CK = 64           # k/v cols per core (1 kv head)
CQK = CQ + CK     # 320
CQKV = CQK + CK   # 384: fused q+k+v natural tile width
NT = N // P       # 16 token tiles
KD = D // P       # 8 contraction chunks
T_, H_, W_ = 8, 16, 16
THETA = 10000.0
EPS = 1e-6
NCORES = 8

_PROGRAM = None  # (nc, input_names) cache
LAST_RESULTS = None  # BassKernelResults of the last run (for test harness)


def _emit(ctx: ExitStack, tc: "tile.TileContext"):
    nc = tc.nc
    f = F32
    bf = BF16

    xT = nc.dram_tensor("xT", [D, N], bf, kind="ExternalInput").ap()
    wqkv = nc.dram_tensor("wqkv", [D, CQKV], bf, kind="ExternalInput").ap()
    biasd = nc.dram_tensor("biasv", [CQKV], bf, kind="ExternalInput").ap()
    wo = nc.dram_tensor("wo", [CQ, D], bf, kind="ExternalInput").ap()
    cosd = nc.dram_tensor("cos5", [N, CQK], bf, kind="ExternalInput").ap()
    sind = nc.dram_tensor("sin5", [N, CQK], bf, kind="ExternalInput").ap()
    identd = nc.dram_tensor("ident", [P, P], bf, kind="ExternalInput").ap()
    yT = nc.dram_tensor("yT", [D, N], f, kind="ExternalOutput").ap()

    # ---------------- persistent pools ----------------
    constp = ctx.enter_context(tc.tile_pool(name="const", bufs=1))
    ident = constp.tile([P, P], bf, tag="ident")
    nc.sync.dma_start(out=ident[:], in_=identd)
    epsb = constp.tile([P, 1], f, tag="epsb")
    nc.vector.memset(epsb[:], EPS)
    ones_r = constp.tile([1, P], bf, tag="ones_r")
    nc.vector.memset(ones_r[:], 1.0)
    bias_sb = constp.tile([1, CQKV], bf, tag="bias_sb")
    nc.sync.dma_start(out=bias_sb[:], in_=biasd.rearrange("(a c) -> a c", a=1))
    wo_sb = constp.tile([P, 2, D], bf, tag="wo")
    wo_t = wo.rearrange("(k p) d -> k p d", p=P)
    for k2 in range(2):
        nc.sync.dma_start(out=wo_sb[:, k2, :], in_=wo_t[k2])

    persist = ctx.enter_context(tc.tile_pool(name="persist", bufs=1))
    # QT2[:, i, :] = Q^T for head pair i (head 2i at partitions 0:64, 2i+1 at 64:128)
    QT2 = persist.tile([P, 2, N], bf, tag="QT2")
    # K^T duplicated on partitions [0:64] and [64:128]
    kT2 = persist.tile([P, N], bf, tag="kT2")
    v_nat = persist.tile([P, NT, HD + 1], bf, tag="v_nat")
    nc.vector.memset(v_nat[:, :, HD:HD + 1], 1.0)
    # O natural [tokens(tile-major), 4 heads * 64]
    O_nat = persist.tile([P, NT, CQ], bf, tag="O_nat")
    # O^T for out-proj: [:, k2, :] = rows of heads (2*k2, 2*k2+1)
    oT = persist.tile([P, 2, N], bf, tag="oT")

    # ================ phase A: proj + norm + rope + transposes ================
    with tc.tile_pool(name="xw", bufs=1) as xwp, \
         tc.tile_pool(name="cs", bufs=1) as csp, \
         tc.tile_pool(name="psA", bufs=2, space="PSUM") as psA, \
         tc.tile_pool(name="psT", bufs=2, space="PSUM") as psT, \
         tc.tile_pool(name="nat", bufs=3) as natp, \
         tc.tile_pool(name="sm", bufs=4) as smp:

        xT_sb = xwp.tile([P, KD, N], bf, tag="xT")
        xT_t = xT.rearrange("(k p) n -> k p n", p=P)
        for k in range(KD):
            nc.sync.dma_start(out=xT_sb[:, k, :], in_=xT_t[k])
        w_sb = xwp.tile([P, KD, CQKV], bf, tag="wqkv")
        w_t = wqkv.rearrange("(k p) m -> k p m", p=P)
        for k in range(KD):
            nc.sync.dma_start(out=w_sb[:, k, :], in_=w_t[k])

        cosd_t = cosd.rearrange("(t p) c -> t p c", p=P)
        sind_t = sind.rearrange("(t p) c -> t p c", p=P)

        for i in range(NT):
            nsl = slice(i * P, (i + 1) * P)
            # ---- fused QKV projection, natural orientation ----
            pa = psA.tile([P, CQKV], f, tag="pa")
            for k in range(KD):
                nc.tensor.matmul(
                    pa[:],
                    lhsT=xT_sb[:, k, nsl],
                    rhs=w_sb[:, k, :],
                    start=(k == 0),
                    stop=False,
                )
            nc.tensor.matmul(
                pa[:], lhsT=ones_r[:], rhs=bias_sb[:], start=False, stop=True,
            )

            qk = natp.tile([P, CQK], bf, tag="qk")
            nc.scalar.copy(qk[:], pa[:, 0:CQK])          # ACT: PSUM -> SBUF bf16
            nc.vector.tensor_copy(v_nat[:, i, 0:HD], pa[:, CQK:CQKV])  # DVE

            cos_t = csp.tile([P, CQK], bf, tag="cos", bufs=6)
            nc.sync.dma_start(out=cos_t[:], in_=cosd_t[i])
            sin_t = csp.tile([P, CQK], bf, tag="sin", bufs=6)
            nc.sync.dma_start(out=sin_t[:], in_=sind_t[i])

            # ---- RMS stats on raw qk (norm applied after rope) ----
            sq = smp.tile([P, CQK], bf, tag="sq")
            nc.gpsimd.tensor_tensor(sq[:], qk[:], qk[:], op=ALU.mult)  # Pool
            ssum = smp.tile([P, 5], f, tag="ssum")
            nc.vector.tensor_reduce(
                ssum[:], sq.rearrange("p (h d) -> p h d", d=HD), axis=AX.X, op=ALU.add
            )
            rms = smp.tile([P, 5], f, tag="rms")
            nc.scalar.activation(rms[:], ssum[:], AF.Sqrt, bias=epsb[:], scale=1.0 / HD)
            rinv = smp.tile([P, 5], f, tag="rinv")
            nc.vector.reciprocal(rinv[:], rms[:])

            # ---- RoPE in pair-interleaved layout ----
            # qkc = qk * cos5 (Pool); rs = swap2(qk) * sin5 (DVE);
            # qkf = (qkc + rs) * rinv (DVE x2)
            qkc = smp.tile([P, CQK], bf, tag="qkc")
            nc.gpsimd.tensor_tensor(qkc[:], qk[:], cos_t[:], op=ALU.mult)
            rs = smp.tile([P, CQK], bf, tag="rs")
            qk_sw = qk.rearrange("p (j s) -> p j s", s=2)[:, :, ::-1]
            nc.vector.tensor_tensor(rs[:], qk_sw, sin_t[:], op=ALU.mult)
            qkf = natp.tile([P, CQK], bf, tag="qkf")
            nc.vector.tensor_tensor(qkf[:], qkc[:], rs[:], op=ALU.add)
            qkf5 = qkf.rearrange("p (h d) -> p h d", d=HD)
            nc.vector.tensor_mul(qkf5, qkf5, rinv.to_broadcast((P, 5, HD)))

            # ---- transpose back to ^T (q pair + k duplicated) ----
            pt2 = psT.tile([P, 512], bf, tag="pt2")
            nc.tensor.transpose(pt2[:, 0:P], qkf[:, 0:P], ident[:])
            nc.tensor.transpose(pt2[:, P:2 * P], qkf[:, P:2 * P], ident[:])
            nc.tensor.transpose(pt2[0:CK, 2 * P:3 * P], qkf[:, 2 * P:2 * P + CK], ident[:])
            nc.tensor.transpose(pt2[CK:2 * CK, 3 * P:4 * P], qkf[:, 2 * P:2 * P + CK], ident[:])
            nc.scalar.copy(
                QT2[:, :, nsl], pt2[:, 0:2 * P].rearrange("p (h n) -> p h n", h=2)
            )
            nc.vector.tensor_copy(kT2[0:CK, nsl], pt2[0:CK, 2 * P:3 * P])
            nc.vector.tensor_copy(kT2[CK:2 * CK, nsl], pt2[CK:2 * CK, 3 * P:4 * P])

    # ================ phase B: attention ================
    scale = float(HD) ** -0.5
    kdebug = bool(os.environ.get("KDEBUG"))
    if kdebug:
        dP16 = nc.dram_tensor("dP16", [NT, P, 1024], BF16, kind="ExternalOutput").ap()
    with tc.tile_pool(name="psS", bufs=2, space="PSUM") as psS, \
         tc.tile_pool(name="psO", bufs=1, space="PSUM") as psO, \
         tc.tile_pool(name="psN", bufs=2, space="PSUM") as psN, \
         tc.tile_pool(name="pT", bufs=3) as pTp, \
         tc.tile_pool(name="ot", bufs=2) as otp, \
         tc.tile_pool(name="rr", bufs=2) as rrp:
        for hh in range(2):          # query half
            nof = hh * 1024
            for h in range(4):       # head
                qrow = (h % 2) * CK
                # O'^T accumulator [65, 1024]: V||ones stationary (cheap
                # amortized weight load), P^T moving 512 cols per matmul
                ops = psO.tile([HD + 1, 1024], f, tag="psO")
                for mc in range(NT):  # key chunk
                    msl = slice(mc * P, (mc + 1) * P)
                    sps = psS.tile([P, 1024], f, tag="psS")
                    for sg in range(2):
                        nc.tensor.matmul(
                            sps[:, sg * 512:(sg + 1) * 512],
                            lhsT=kT2[qrow:qrow + CK, msl],
                            rhs=QT2[qrow:qrow + CK, h // 2,
                                    nof + sg * 512:nof + (sg + 1) * 512],
                            start=True,
                            stop=True,
                        )
                    pTt = pTp.tile([P, 1024], bf, tag="pT")
                    nc.scalar.activation(pTt[:], sps[:], AF.Exp, scale=scale)
                    if kdebug and hh == 0 and h == 0:
                        nc.sync.dma_start(out=dP16[mc], in_=pTt[:])
                    for sg in range(2):
                        nc.tensor.matmul(
                            ops[:, sg * 512:(sg + 1) * 512],
                            lhsT=v_nat[:, mc, :],
                            rhs=pTt[:, sg * 512:(sg + 1) * 512],
                            start=(mc == 0),
                            stop=(mc == NT - 1),
                        )
                # O'^T -> SBUF bf16, transpose to natural, normalize per-q
                o_tmp = otp.tile([HD + 1, 1024], bf, tag="o_tmp")
                nc.vector.tensor_copy(o_tmp[:], ops[:])
                ptn = psN.tile([P, 8, HD + 2], bf, tag="ptn")
                for qb in range(8):
                    nc.tensor.transpose(
                        ptn[:, qb, 0:HD + 1],
                        o_tmp[:, qb * P:(qb + 1) * P],
                        ident[0:HD + 1, 0:HD + 1],
                    )
                rin = rrp.tile([P, 8], f, tag="rin")
                nc.vector.reciprocal(rin[:], ptn[:, :, HD])
                nc.vector.tensor_mul(
                    O_nat[:, hh * 8:(hh + 1) * 8, h * HD:(h + 1) * HD],
                    ptn[:, :, 0:HD],
                    rin.to_broadcast((P, 8, HD)),
                )

    # ================ phase B2: O_nat -> O^T ================
    with tc.tile_pool(name="psU", bufs=2, space="PSUM") as psU, \
         tc.tile_pool(name="ub", bufs=2) as ubp:
        for i in range(NT):
            ptu = psU.tile([P, 2, P], bf, tag="ptu")
            for k2 in range(2):
                nc.tensor.transpose(
                    ptu[:, k2, :], O_nat[:, i, k2 * P:(k2 + 1) * P], ident[:]
                )
            nc.vector.tensor_copy(oT[:, :, i * P:(i + 1) * P], ptu[:])

    if os.environ.get("KDEBUG"):
        for nm, t in (("dQT2", QT2), ("dkT2", kT2), ("dvnat", v_nat),
                      ("dOnat", O_nat), ("doT", oT)):
            shp = [P] + list(t.shape[1:])
            dd = nc.dram_tensor(nm, shp, BF16, kind="ExternalOutput").ap()
            nc.sync.dma_start(out=dd, in_=t[:])

    # ================ phase C: partial out-proj y^T = Wo_g.T @ O_norm^T ======
    with tc.tile_pool(name="psY", bufs=2, space="PSUM") as psY, \
         tc.tile_pool(name="ysb", bufs=2) as ysbp:
        yT_t = yT.rearrange("(t p) n -> t p n", p=P)
        for mt in range(8):
            yps = psY.tile([P, N], f, tag="psY")
            for k2 in range(2):
                for sg in range(4):
                    nc.tensor.matmul(
                        yps[:, sg * 512:(sg + 1) * 512],
                        lhsT=wo_sb[:, k2, mt * P:(mt + 1) * P],
                        rhs=oT[:, k2, sg * 512:(sg + 1) * 512],
                        start=(k2 == 0),
                        stop=(k2 == 1),
                    )
            ysb = ysbp.tile([P, N], f, tag="ysb")
            nc.vector.tensor_copy(ysb[:], yps[:])
            nc.sync.dma_start(out=yT_t[mt], in_=ysb[:])


def _build_program():
    global _PROGRAM
    if _PROGRAM is not None:
        return _PROGRAM
    nc = bacc.Bacc(
        "TRN2",
        target_bir_lowering=False,
        debug=False,
        enable_asserts=False,
        num_devices=NCORES,
    )
    with tile.TileContext(nc) as tc:
        with ExitStack() as ctx:
            _emit(ctx, tc)
    nc.finalize()
    _PROGRAM = nc
    return nc


# ---------------- host-side RoPE/scale table construction ----------------

def _rope_cs(n, d):
    # bit-identical to reference._rope_cs
    inv = 1.0 / (THETA ** (np.arange(0, d, 2, dtype=np.float32) / d))
    fr = np.arange(n, dtype=np.float32)[:, None] * inv[None, :]
    emb = np.concatenate([fr, fr], axis=-1)
    return np.cos(emb), np.sin(emb)


def _perm():
    dt = HD // 4          # 16
    dh = HD // 4          # 16
    dw = HD - dt - dh     # 32
    perm = np.empty(HD, np.int64)
    for off, sz in ((0, dt), (dt, dh), (dt + dh, dw)):
        m = sz // 2
        for j in range(m):
            perm[off + 2 * j] = off + j
            perm[off + 2 * j + 1] = off + m + j
    return perm


def _host_tables(qn_w, kn_w):
    dt = HD // 4
    dh = HD // 4
    dw = HD - dt - dh
    cos_t, sin_t = _rope_cs(T_, dt)
    cos_h, sin_h = _rope_cs(H_, dh)
    cos_w, sin_w = _rope_cs(W_, dw)
    tt = np.repeat(np.arange(T_), H_ * W_)
    hh = np.tile(np.repeat(np.arange(H_), W_), T_)
    ww = np.tile(np.arange(W_), T_ * H_)
    cos = np.empty((N, HD), np.float32)
    sin = np.empty((N, HD), np.float32)
    cos[:, 0:dt] = cos_t[tt]
    cos[:, dt:dt + dh] = cos_h[hh]
    cos[:, dt + dh:] = cos_w[ww]
    sin[:, 0:dt] = sin_t[tt]
    sin[:, dt:dt + dh] = sin_h[hh]
    sin[:, dt + dh:] = sin_w[ww]

    perm = _perm()
    cosP = cos[:, perm]
    sgn = np.empty(HD, np.float32)
    sgn[0::2] = -1.0
    sgn[1::2] = 1.0
    sgnsinP = sin[:, perm] * sgn[None, :]

    def fold(w):
        wp = np.asarray(w, np.float32)[perm]
        swp = wp.reshape(-1, 2)[:, ::-1].reshape(-1)
        return cosP * wp[None, :], sgnsinP * swp[None, :]

    cos_q, sin_q = fold(qn_w)
    cos_k, sin_k = fold(kn_w)
    cos5 = np.concatenate([np.tile(cos_q, (1, 4)), cos_k], axis=1)
    sin5 = np.concatenate([np.tile(sin_q, (1, 4)), sin_k], axis=1)
    return np.ascontiguousarray(cos5), np.ascontiguousarray(sin5)


def _bf16(a):
    return np.asarray(a, np.float32).astype(ml_dtypes.bfloat16)


def kernel(**inputs):
    global LAST_RESULTS
    x = np.asarray(inputs["x"], np.float32)
    Wq = np.asarray(inputs["Wq"], np.float32)
    Wk = np.asarray(inputs["Wk"], np.float32)
    Wv = np.asarray(inputs["Wv"], np.float32)
    Wo = np.asarray(inputs["Wo"], np.float32)
    bq = np.asarray(inputs["bq"], np.float32)
    bk = np.asarray(inputs["bk"], np.float32)
    bv = np.asarray(inputs["bv"], np.float32)
    bo = np.asarray(inputs["bo"], np.float32)
    qn_w = np.asarray(inputs["qn_w"], np.float32)
    kn_w = np.asarray(inputs["kn_w"], np.float32)

    assert x.shape == (B, N, D), x.shape
    cos5, sin5 = _host_tables(qn_w, kn_w)
    perm = _perm()

    nc = _build_program()
    in_maps = []
    xT_b = [np.ascontiguousarray(_bf16(x[b]).T) for b in range(B)]
    identb = np.eye(P, dtype=ml_dtypes.bfloat16)
    cos5b = _bf16(cos5)
    sin5b = _bf16(sin5)
    for c in range(NCORES):
        b, g = c // 4, c % 4
        wq_g = Wq[:, g * CQ:(g + 1) * CQ].reshape(D, 4, HD)[:, :, perm].reshape(D, CQ)
        wk_g = Wk[:, g * CK:(g + 1) * CK][:, perm]
        wv_g = Wv[:, g * CK:(g + 1) * CK]
        wqkv = np.concatenate([wq_g, wk_g, wv_g], axis=1)
        bq_g = bq[g * CQ:(g + 1) * CQ].reshape(4, HD)[:, perm].reshape(CQ)
        bk_g = bk[g * CK:(g + 1) * CK][perm]
        bias = np.concatenate([bq_g, bk_g, bv[g * CK:(g + 1) * CK]])
        in_maps.append({
            "xT": xT_b[b],
            "wqkv": np.ascontiguousarray(_bf16(wqkv)),
            "biasv": np.ascontiguousarray(_bf16(bias)),
            "wo": np.ascontiguousarray(_bf16(Wo[g * CQ:(g + 1) * CQ, :])),
            "cos5": cos5b,
            "sin5": sin5b,
            "ident": identb,
        })

    res = run_bass_kernel_spmd(nc, in_maps, list(range(NCORES)))
    LAST_RESULTS = res
    out = np.empty((B, N, D), np.float32)
    for b in range(B):
        acc = res.results[4 * b]["yT"].astype(np.float32)
        for g in range(1, 4):
            acc = acc + res.results[4 * b + g]["yT"]
        out[b] = acc.T + bo[None, :]
    return out


if __name__ == "__main__":
    # smoke: build only
    nc = _build_program()
    print("built ok")
